# revision 18
# baseline (speedup 1.0000x reference)
"""FocalLoss + MDCA loss kernel for TRN2, 8-core data-parallel.

reference:
    loss_cls = mean_i[-(1-pt_i) * log(pt_i)],  pt_i = probs[i, targets[i]]
    loss_cal = mean_c |mean_i probs[i,c] - count_c/B|
    out = loss_cls + loss_cal        (GAMMA=1, BETA=1)

Strategy: shard batch (16384) across 8 cores (2048 rows each). Each core:
  - streams its probs shard HBM->SBUF with an inline fp32->fp16 cast (SWDGE);
    all 16 tile DMAs are emitted up front into dedicated buffers
  - PE matmul ones[128,1]^T @ probs_fp16 accumulates column sums in PSUM (fp32)
  - DVE builds one-hot rows eq[p,c] = (c == target_p) from an iota constant,
    PE matmul ones^T @ eq accumulates the target histogram in PSUM (exact)
  - pt extraction: DVE multiplies probs*eq, ACT Copy-with-accumulate row-sums
    it (exact gather of the fp16-quantized prob); ACT computes Ln, DVE fuses
    (pt-1)*ln(pt) with a row-sum, PE transposes the [128] focal partials and
    ACT reduces them to one scalar
  - everything lands in ONE [1, 2001] f32 output row -> a single contiguous
    DMA (per-partition 4B writes to DRAM pay a ~9us RMW receipt)
Host combines the 8 cores' colsum/hist/focal partials into the scalar loss
(the gather/unshard step).

The walrus build in this env encodes at most ONE sync wait per instruction;
_split_multi_waits post-processes the scheduled program to hoist extra waits
onto same-engine EventSemaphore carriers.
"""

import numpy as np

import concourse.bass as bass
import concourse.mybir as mybir
import concourse.tile as tile
from concourse.bass_utils import run_bass_kernel_spmd

B, C = 16384, 1000
NCORES = 8
BC = B // NCORES  # 2048 rows per core
P = 128
NT = BC // P      # 16 batch tiles per core
CH = 500          # matmul half free-dim (PSUM bank = 512 fp32)
OUT_W = 2001      # [colsum 0:1000 | hist 1000:2000 | focal_sum 2000]

F32 = mybir.dt.float32
F16 = mybir.dt.float16
I16 = mybir.dt.int16
I32 = mybir.dt.int32


def emit_kernel(ctx, tc, probs_d, targ_d, out_d):
    nc = tc.nc
    Alu = mybir.AluOpType
    from concourse.masks import make_identity

    consts = ctx.enter_context(tc.tile_pool(name="consts", bufs=1))
    # bufs=NT: every probs tile gets its own SBUF slot so no pf16 DMA carries
    # a slot-reuse wait and the DMA queue can run the whole shard untangled.
    probs_pool = ctx.enter_context(tc.tile_pool(name="probs_pool", bufs=NT))
    eq_pool = ctx.enter_context(tc.tile_pool(name="eq_pool", bufs=NT))
    psum = ctx.enter_context(tc.tile_pool(name="psum", bufs=1, space="PSUM"))

    # 1) first two probs tile loads start immediately (SDMA drains them while
    # POOL builds the constants); the rest are emitted right after.
    def load_tile(i):
        pf16 = probs_pool.tile([P, C], F16, tag="pf16", name=f"pf16_{i}")
        nc.gpsimd.dma_start(out=pf16[:], in_=probs_d[i * P:(i + 1) * P, :])
        return pf16

    pf16s = [load_tile(0), load_tile(1)]

    # 2) targets: one [16, 128] contiguous load (HWDGE), PE-transpose to
    # [128, 16] so tile i's targets sit at column i as per-partition scalars.
    t_rows_i32 = consts.tile([NT, P], I32, tag="t_rows_i32")
    nc.sync.dma_start(out=t_rows_i32[:], in_=targ_d.rearrange("(i p) -> i p", p=P))

    # 3) constants
    ones = consts.tile([P, 1], F16, tag="ones")
    nc.vector.memset(ones[:], 1.0)
    iota_i16 = consts.tile([P, C], I16, tag="iota_i16")
    nc.gpsimd.iota(iota_i16[:], pattern=[[1, C]], base=0, channel_multiplier=0)
    identity = consts.tile([P, P], F32, tag="identity")
    make_identity(nc, identity[:])
    # row indices for the pt gather: rows[p, j] = 128j + p (iota pattern
    # steps must fit int16, so scale by C on DVE afterwards)
    rows_i32 = consts.tile([P, NT], I32, tag="rows_i32")
    nc.gpsimd.iota(rows_i32[:], pattern=[[P, NT]], base=0,
                   channel_multiplier=1)

    t_rows_f32 = consts.tile([NT, P], F32, tag="t_rows_f32")
    # gpsimd (not DVE) so the PE transpose below has single-engine producers
    nc.gpsimd.tensor_copy(t_rows_f32[:], t_rows_i32[:])
    t_ps = psum.tile([P, NT], F32, tag="t_ps")
    nc.tensor.transpose(t_ps[:], t_rows_f32[:], identity[:NT, :NT])
    t_cols = consts.tile([P, NT], F32, tag="t_cols")
    nc.vector.tensor_copy(t_cols[:], t_ps[:])
    t_cols_i32 = consts.tile([P, NT], I32, tag="t_cols_i32")
    nc.vector.tensor_copy(t_cols_i32[:], t_ps[:])

    # pt[p, j] = probs[128j + p, t]: element offsets for per-tile [128, 1]
    # indirect gathers (a 2048-descriptor single gather mis-fetched a few
    # elements on HW; the [128, 1] shape is the production-proven pattern)
    offs = consts.tile([P, NT], I32, tag="offs")
    nc.vector.tensor_scalar(out=offs[:], in0=rows_i32[:], scalar1=float(C),
                            scalar2=None, op0=Alu.mult)
    nc.vector.tensor_tensor(out=offs[:], in0=offs[:], in1=t_cols_i32[:],
                            op=Alu.add)
    pt_all = consts.tile([P, NT], F32, tag="pt_all")
    probs_flat = probs_d.rearrange("a b -> (a b)")[:, None]

    def gather_tile(i):
        nc.gpsimd.indirect_dma_start(
            out=pt_all[:, i:i + 1], out_offset=None,
            in_=probs_flat,
            in_offset=bass.IndirectOffsetOnAxis(ap=offs[:, i:i + 1], axis=0),
        )

    # remaining probs loads, with the pt gathers threaded between them on
    # POOL so neither delays the other's descriptor emission stream
    for i in range(2, NT):
        pf16s.append(load_tile(i))
        if i >= 4:
            gather_tile(i - 4)
    for i in range(NT - 4, NT):
        gather_tile(i)

    # persistent accumulators
    cs_ps = [psum.tile([1, CH], F32, tag=f"cs_ps{h}", name=f"cs_ps{h}")
             for h in range(2)]
    hs_ps = [psum.tile([1, CH], F32, tag=f"hs_ps{h}", name=f"hs_ps{h}")
             for h in range(2)]

    # 4) main loop
    for i in range(NT):
        pf16 = pf16s[i]
        # one-hot rows: eq[p, c] = (c == target_p)
        eq = eq_pool.tile([P, C], F16, tag="eq", name=f"eq_{i}")
        nc.vector.tensor_scalar(
            out=eq[:], in0=iota_i16[:], scalar1=t_cols[:, i:i + 1], scalar2=None,
            op0=Alu.is_equal,
        )

        first, last = (i == 0), (i == NT - 1)
        for h in range(2):
            sl = slice(h * CH, (h + 1) * CH)
            nc.tensor.matmul(cs_ps[h][:], ones[:], pf16[:, sl],
                             start=first, stop=last)
            nc.tensor.matmul(hs_ps[h][:], ones[:], eq[:, sl],
                             start=first, stop=last)


    # 5) focal tail: focal[p] = sum_i (pt - 1) * ln(pt).
    # Stage [pt | ln(pt)] side by side via ACT so the DVE reduce depends on a
    # single engine.
    pl = consts.tile([P, 2 * NT], F32, tag="pl")
    nc.scalar.copy(pl[:, 0:NT], pt_all[:])
    nc.scalar.activation(pl[:, NT:2 * NT], pt_all[:],
                         mybir.ActivationFunctionType.Ln)
    junk = consts.tile([P, NT], F32, tag="junk")
    focal = consts.tile([P, 1], F32, tag="focal")
    nc.vector.scalar_tensor_tensor(
        out=junk[:], in0=pl[:, 0:NT], scalar=1.0, in1=pl[:, NT:2 * NT],
        op0=Alu.subtract, op1=Alu.mult, accum_out=focal[:],
    )
    # reduce focal over partitions: PE transpose to a row, ACT accumulates
    fc_t = psum.tile([1, P], F32, tag="fc_t")
    nc.tensor.transpose(fc_t[:], focal[:], identity[:])

    # 6) pack [colsum | hist | focal_sum] into one row, single output DMA.
    # colsum halves drain on DVE, hist halves on ACT — parallel tail.
    out_sb = consts.tile([1, OUT_W], F32, tag="out_sb")
    for h in range(2):
        nc.vector.tensor_copy(out_sb[:, h * CH:(h + 1) * CH], cs_ps[h][:])
        nc.scalar.copy(out_sb[:, C + h * CH:C + (h + 1) * CH], hs_ps[h][:])
    fc_row = consts.tile([1, P], F32, tag="fc_row")
    nc.scalar.activation(fc_row[:], fc_t[:],
                         mybir.ActivationFunctionType.Copy,
                         accum_out=out_sb[:, 2 * C:2 * C + 1])
    nc.sync.dma_start(out=out_d[:, :], in_=out_sb[:])


def _split_multi_waits(nc):
    """The walrus build in this env encodes at most ONE sync wait per
    instruction (newer Tile emits several, e.g. on its tail drain). Hoist
    extra waits onto EventSemaphore carrier instructions inserted just
    before, on the same engine — same-engine program order makes this
    semantically identical."""
    n = 0
    for f in nc.m.functions:
        for blk in f.blocks:
            il = blk.instructions
            i = 0
            while i < len(il):
                inst = il[i]
                si = inst.sync_info
                ws = list(si.on_wait) if si is not None else []
                if len(ws) > 1:
                    for w in ws[:-1]:
                        ev = mybir.InstEventSemaphore(
                            name=f"I-waitsplit-{n}", ins=[], outs=[])
                        n += 1
                        ev.engine = inst.engine
                        ev.sync_info = mybir.SyncInfo(on_wait=[w], on_update=[])
                        il.insert(i, ev)
                        i += 1
                    inst.sync_info = mybir.SyncInfo(
                        on_wait=[ws[-1]], on_update=list(si.on_update))
                i += 1


_cached_nc = {}


def build_nc(split_waits=True):
    global _cached_nc
    if split_waits in _cached_nc:
        return _cached_nc[split_waits]
    from contextlib import ExitStack

    nc = bass.Bass("TRN2")
    probs_d = nc.dram_tensor("probs", [BC, C], F32, kind="ExternalInput").ap()
    targ_d = nc.dram_tensor("targets", [BC], I32, kind="ExternalInput").ap()
    out_d = nc.dram_tensor("out_all", [1, OUT_W], F32, kind="ExternalOutput").ap()

    with tile.TileContext(nc) as tc:
        with ExitStack() as ctx:
            emit_kernel(ctx, tc, probs_d, targ_d, out_d)
    if split_waits:
        _split_multi_waits(nc)
    _cached_nc[split_waits] = nc
    return nc


def make_in_maps(probs, targets):
    probs = np.ascontiguousarray(np.asarray(probs), dtype=np.float32)
    targets = np.asarray(targets).astype(np.int32)
    assert probs.shape == (B, C) and targets.shape == (B,)
    return [
        {
            "probs": probs[k * BC:(k + 1) * BC],
            "targets": np.ascontiguousarray(targets[k * BC:(k + 1) * BC]),
        }
        for k in range(NCORES)
    ]


def combine(results):
    cs = np.zeros(C, np.float64)
    hs = np.zeros(C, np.float64)
    fc = 0.0
    for r in results:
        row = r["out_all"].reshape(OUT_W).astype(np.float64)
        cs += row[0:C]
        hs += row[C:2 * C]
        fc += row[2 * C]
    loss_cls = fc / B
    loss_cal = float(np.mean(np.abs(cs / B - hs / B)))
    return np.asarray(loss_cls + 1.0 * loss_cal, dtype=np.float32)


def run_spmd(probs, targets, **kwargs):
    nc = build_nc()
    in_maps = make_in_maps(probs, targets)
    return run_bass_kernel_spmd(nc, in_maps, list(range(NCORES)), **kwargs)


def kernel(probs, targets):
    res = run_spmd(probs, targets)
    return combine(res.results)


# revision 20
# speedup vs baseline: 1.1486x; 1.1486x over previous
"""FocalLoss + MDCA loss kernel for TRN2, 8-core data-parallel.

reference:
    loss_cls = mean_i[-(1-pt_i) * log(pt_i)],  pt_i = probs[i, targets[i]]
    loss_cal = mean_c |mean_i probs[i,c] - count_c/B|
    out = loss_cls + loss_cal        (GAMMA=1, BETA=1)

Strategy: shard batch (16384) across 8 cores (2048 rows each). Each core:
  - streams its probs shard HBM->SBUF with an inline fp32->fp16 cast (SWDGE)
    into dedicated per-tile buffers (no slot-reuse waits on the DMAs)
  - PE matmul ones[128,1]^T @ probs_fp16 accumulates column sums in PSUM (fp32)
  - DVE builds one-hot rows eq[p,c] = (c == target_p) from an iota constant,
    PE matmul ones^T @ eq accumulates the target histogram in PSUM (exact)
  - pt[p] = sum_c probs*eq (exact gather of the fp16-quantized prob), split
    across engines: most tiles DVE-mult + ACT Copy-accumulate, the rest a
    fused DVE scalar_tensor_tensor reduce
  - ACT stages [pt | ln(pt)], DVE fuses (pt-1)*ln(pt) with a row-sum, PE
    transposes the [128] focal partials and ACT reduces them to one scalar
  - everything lands in ONE [1, 2001] f32 output row -> a single contiguous
    DMA (per-partition 4B writes to DRAM pay a ~9us RMW receipt)
Host combines the 8 cores' colsum/hist/focal partials into the scalar loss
(the gather/unshard step).

The walrus build in this env encodes at most ONE sync wait per instruction;
_split_multi_waits post-processes the scheduled program to hoist extra waits
onto same-engine EventSemaphore carriers.
"""

import numpy as np

import concourse.bass as bass
import concourse.mybir as mybir
import concourse.tile as tile
from concourse.bass_utils import run_bass_kernel_spmd

B, C = 16384, 1000
NCORES = 8
BC = B // NCORES  # 2048 rows per core
P = 128
NT = BC // P      # 16 batch tiles per core
CH = 500          # matmul half free-dim (PSUM bank = 512 fp32)
OUT_W = 2001      # [colsum 0:1000 | hist 1000:2000 | focal_sum 2000]

F32 = mybir.dt.float32
F16 = mybir.dt.float16
I16 = mybir.dt.int16
I32 = mybir.dt.int32


def emit_kernel(ctx, tc, probs_d, targ_d, out_d):
    nc = tc.nc
    Alu = mybir.AluOpType
    from concourse.masks import make_identity

    consts = ctx.enter_context(tc.tile_pool(name="consts", bufs=1))
    probs_pool = ctx.enter_context(tc.tile_pool(name="probs_pool", bufs=NT))
    eq_pool = ctx.enter_context(tc.tile_pool(name="eq_pool", bufs=NT))
    msk_pool = ctx.enter_context(tc.tile_pool(name="msk_pool", bufs=NT))
    ajunk_pool = ctx.enter_context(tc.tile_pool(name="ajunk_pool", bufs=2))
    psum = ctx.enter_context(tc.tile_pool(name="psum", bufs=1, space="PSUM"))

    # 1) first two probs tile loads start immediately (SDMA drains them while
    # POOL builds the constants); the rest are emitted right after.
    def load_tile(i):
        pf16 = probs_pool.tile([P, C], F16, tag="pf16", name=f"pf16_{i}")
        nc.gpsimd.dma_start(out=pf16[:], in_=probs_d[i * P:(i + 1) * P, :])
        return pf16

    pf16s = [load_tile(0), load_tile(1)]

    # 2) targets: one [16, 128] contiguous load (HWDGE), PE-transpose to
    # [128, 16] so tile i's targets sit at column i as per-partition scalars.
    t_rows_i32 = consts.tile([NT, P], I32, tag="t_rows_i32")
    nc.sync.dma_start(out=t_rows_i32[:], in_=targ_d.rearrange("(i p) -> i p", p=P))

    # 3) constants
    ones = consts.tile([P, 1], F16, tag="ones")
    nc.vector.memset(ones[:], 1.0)
    iota_i16 = consts.tile([P, C], I16, tag="iota_i16")
    nc.gpsimd.iota(iota_i16[:], pattern=[[1, C]], base=0, channel_multiplier=0)
    iota_f16 = consts.tile([P, C], F16, tag="iota_f16")
    nc.vector.tensor_copy(iota_f16[:], iota_i16[:])
    identity = consts.tile([P, P], F32, tag="identity")
    make_identity(nc, identity[:])

    t_rows_f32 = consts.tile([NT, P], F32, tag="t_rows_f32")
    # gpsimd (not DVE) so the PE transpose below has single-engine producers
    nc.gpsimd.tensor_copy(t_rows_f32[:], t_rows_i32[:])
    t_ps = psum.tile([P, NT], F32, tag="t_ps")
    nc.tensor.transpose(t_ps[:], t_rows_f32[:], identity[:NT, :NT])
    t_cols = consts.tile([P, NT], F32, tag="t_cols")
    nc.vector.tensor_copy(t_cols[:], t_ps[:])

    # remaining probs loads
    pf16s += [load_tile(i) for i in range(2, NT)]

    # persistent accumulators
    cs_ps = [psum.tile([1, CH], F32, tag=f"cs_ps{h}", name=f"cs_ps{h}")
             for h in range(2)]
    hs_ps = [psum.tile([1, CH], F32, tag=f"hs_ps{h}", name=f"hs_ps{h}")
             for h in range(2)]
    pt_all = consts.tile([P, NT], F32, tag="pt_all")

    # 4) main loop
    for i in range(NT):
        pf16 = pf16s[i]
        # one-hot rows: eq[p, c] = (c == target_p)
        eq = eq_pool.tile([P, C], F16, tag="eq", name=f"eq_{i}")
        nc.vector.tensor_scalar(
            out=eq[:], in0=iota_f16[:], scalar1=t_cols[:, i:i + 1], scalar2=None,
            op0=Alu.is_equal,
        )

        first, last = (i == 0), (i == NT - 1)
        for h in range(2):
            sl = slice(h * CH, (h + 1) * CH)
            nc.tensor.matmul(cs_ps[h][:], ones[:], pf16[:, sl],
                             start=first, stop=last)
            nc.tensor.matmul(hs_ps[h][:], ones[:], eq[:, sl],
                             start=first, stop=last)

        # pt[p] = sum_c probs[p,c] * eq[p,c] — an exact one-element gather.
        # Split across engines to keep both under the DMA roofline: most
        # tiles go DVE-mult + ACT Copy-accumulate; the rest use the fused
        # DVE scalar_tensor_tensor reduce.
        if i % 8 < 5:
            msk = msk_pool.tile([P, C], F16, tag="msk", name=f"msk_{i}")
            nc.vector.tensor_tensor(out=msk[:], in0=eq[:], in1=pf16[:],
                                    op=Alu.mult)
            ajunk = ajunk_pool.tile([P, C], F16, tag="ajunk")
            nc.scalar.activation(ajunk[:], msk[:],
                                 mybir.ActivationFunctionType.Copy,
                                 accum_out=pt_all[:, i:i + 1])
        else:
            msk = msk_pool.tile([P, C], F16, tag="msk", name=f"msk_{i}")
            nc.vector.scalar_tensor_tensor(
                out=msk[:], in0=iota_f16[:], scalar=t_cols[:, i:i + 1],
                in1=pf16[:], op0=Alu.is_equal, op1=Alu.mult,
                accum_out=pt_all[:, i:i + 1],
            )

    # 5) focal tail: focal[p] = sum_i (pt - 1) * ln(pt).
    # Stage [pt | ln(pt)] side by side via ACT so the DVE reduce depends on a
    # single engine.
    pl = consts.tile([P, 2 * NT], F32, tag="pl")
    nc.scalar.copy(pl[:, 0:NT], pt_all[:])
    nc.scalar.activation(pl[:, NT:2 * NT], pt_all[:],
                         mybir.ActivationFunctionType.Ln)
    junk = consts.tile([P, NT], F32, tag="junk")
    focal = consts.tile([P, 1], F32, tag="focal")
    nc.vector.scalar_tensor_tensor(
        out=junk[:], in0=pl[:, 0:NT], scalar=1.0, in1=pl[:, NT:2 * NT],
        op0=Alu.subtract, op1=Alu.mult, accum_out=focal[:],
    )
    # reduce focal over partitions: PE transpose to a row, ACT accumulates
    fc_t = psum.tile([1, P], F32, tag="fc_t")
    nc.tensor.transpose(fc_t[:], focal[:], identity[:])

    # 6) pack [colsum | hist | focal_sum] into one row, single output DMA.
    # colsum halves drain on DVE, hist halves on ACT — parallel tail.
    out_sb = consts.tile([1, OUT_W], F32, tag="out_sb")
    for h in range(2):
        nc.vector.tensor_copy(out_sb[:, h * CH:(h + 1) * CH], cs_ps[h][:])
        nc.scalar.copy(out_sb[:, C + h * CH:C + (h + 1) * CH], hs_ps[h][:])
    fc_row = consts.tile([1, P], F32, tag="fc_row")
    nc.scalar.activation(fc_row[:], fc_t[:],
                         mybir.ActivationFunctionType.Copy,
                         accum_out=out_sb[:, 2 * C:2 * C + 1])
    nc.sync.dma_start(out=out_d[:, :], in_=out_sb[:])


def _split_multi_waits(nc):
    """The walrus build in this env encodes at most ONE sync wait per
    instruction (newer Tile emits several, e.g. on its tail drain). Hoist
    extra waits onto EventSemaphore carrier instructions inserted just
    before, on the same engine — same-engine program order makes this
    semantically identical."""
    n = 0
    for f in nc.m.functions:
        for blk in f.blocks:
            il = blk.instructions
            i = 0
            while i < len(il):
                inst = il[i]
                si = inst.sync_info
                ws = list(si.on_wait) if si is not None else []
                if len(ws) > 1:
                    for w in ws[:-1]:
                        ev = mybir.InstEventSemaphore(
                            name=f"I-waitsplit-{n}", ins=[], outs=[])
                        n += 1
                        ev.engine = inst.engine
                        ev.sync_info = mybir.SyncInfo(on_wait=[w], on_update=[])
                        il.insert(i, ev)
                        i += 1
                    inst.sync_info = mybir.SyncInfo(
                        on_wait=[ws[-1]], on_update=list(si.on_update))
                i += 1


_cached_nc = {}


def build_nc(split_waits=True):
    global _cached_nc
    if split_waits in _cached_nc:
        return _cached_nc[split_waits]
    from contextlib import ExitStack

    nc = bass.Bass("TRN2")
    probs_d = nc.dram_tensor("probs", [BC, C], F32, kind="ExternalInput").ap()
    targ_d = nc.dram_tensor("targets", [BC], I32, kind="ExternalInput").ap()
    out_d = nc.dram_tensor("out_all", [1, OUT_W], F32, kind="ExternalOutput").ap()

    with tile.TileContext(nc) as tc:
        with ExitStack() as ctx:
            emit_kernel(ctx, tc, probs_d, targ_d, out_d)
    if split_waits:
        _split_multi_waits(nc)
    _cached_nc[split_waits] = nc
    return nc


def make_in_maps(probs, targets):
    probs = np.ascontiguousarray(np.asarray(probs), dtype=np.float32)
    targets = np.asarray(targets).astype(np.int32)
    assert probs.shape == (B, C) and targets.shape == (B,)
    return [
        {
            "probs": probs[k * BC:(k + 1) * BC],
            "targets": np.ascontiguousarray(targets[k * BC:(k + 1) * BC]),
        }
        for k in range(NCORES)
    ]


def combine(results):
    cs = np.zeros(C, np.float64)
    hs = np.zeros(C, np.float64)
    fc = 0.0
    for r in results:
        row = r["out_all"].reshape(OUT_W).astype(np.float64)
        cs += row[0:C]
        hs += row[C:2 * C]
        fc += row[2 * C]
    loss_cls = fc / B
    loss_cal = float(np.mean(np.abs(cs / B - hs / B)))
    return np.asarray(loss_cls + 1.0 * loss_cal, dtype=np.float32)


def run_spmd(probs, targets, **kwargs):
    nc = build_nc()
    in_maps = make_in_maps(probs, targets)
    return run_bass_kernel_spmd(nc, in_maps, list(range(NCORES)), **kwargs)


def kernel(probs, targets):
    res = run_spmd(probs, targets)
    return combine(res.results)


# revision 21
# speedup vs baseline: 1.2684x; 1.1042x over previous
"""FocalLoss + MDCA loss kernel for TRN2, 8-core data-parallel.

reference:
    loss_cls = mean_i[-(1-pt_i) * log(pt_i)],  pt_i = probs[i, targets[i]]
    loss_cal = mean_c |mean_i probs[i,c] - count_c/B|
    out = loss_cls + loss_cal        (GAMMA=1, BETA=1)

Strategy: shard batch (16384) across 8 cores (2048 rows each). Each core:
  - streams its probs shard HBM->SBUF with an inline fp32->fp16 cast (SWDGE)
    into dedicated per-tile buffers (no slot-reuse waits on the DMAs)
  - PE matmul ones[128,1]^T @ probs_fp16 accumulates column sums in PSUM (fp32)
  - DVE builds one-hot rows eq[p,c] = (c == target_p) from an iota constant,
    PE matmul ones^T @ eq accumulates the target histogram in PSUM (exact)
  - pt[p] = sum_c probs*eq (exact gather of the fp16-quantized prob), split
    across engines: most tiles DVE-mult + ACT Copy-accumulate, the rest a
    fused DVE scalar_tensor_tensor reduce
  - ACT stages [pt | ln(pt)], DVE fuses (pt-1)*ln(pt) with a row-sum, PE
    transposes the [128] focal partials and ACT reduces them to one scalar
  - everything lands in ONE [1, 2001] f32 output row -> a single contiguous
    DMA (per-partition 4B writes to DRAM pay a ~9us RMW receipt)
Host combines the 8 cores' colsum/hist/focal partials into the scalar loss
(the gather/unshard step).

The walrus build in this env encodes at most ONE sync wait per instruction;
_split_multi_waits post-processes the scheduled program to hoist extra waits
onto same-engine EventSemaphore carriers.
"""

import numpy as np

import concourse.bass as bass
import concourse.mybir as mybir
import concourse.tile as tile
from concourse.bass_utils import run_bass_kernel_spmd

B, C = 16384, 1000
NCORES = 8
BC = B // NCORES  # 2048 rows per core
P = 128
NT = BC // P      # 16 batch tiles per core
CH = 500          # matmul half free-dim (PSUM bank = 512 fp32)
OUT_W = 2001      # [colsum 0:1000 | hist 1000:2000 | focal_sum 2000]

F32 = mybir.dt.float32
F16 = mybir.dt.float16
I16 = mybir.dt.int16
I32 = mybir.dt.int32


def emit_kernel(ctx, tc, probs_d, targ_d, out_d):
    nc = tc.nc
    Alu = mybir.AluOpType
    from concourse.masks import make_identity

    consts = ctx.enter_context(tc.tile_pool(name="consts", bufs=1))
    probs_pool = ctx.enter_context(tc.tile_pool(name="probs_pool", bufs=NT))
    eq_pool = ctx.enter_context(tc.tile_pool(name="eq_pool", bufs=NT))
    msk_pool = ctx.enter_context(tc.tile_pool(name="msk_pool", bufs=NT))
    ajunk_pool = ctx.enter_context(tc.tile_pool(name="ajunk_pool", bufs=2))
    psum = ctx.enter_context(tc.tile_pool(name="psum", bufs=1, space="PSUM"))

    # 1) first two probs tile loads start immediately (SDMA drains them while
    # POOL builds the constants); the rest are emitted right after.
    def load_tile(i):
        pf16 = probs_pool.tile([P, C], F16, tag="pf16", name=f"pf16_{i}")
        nc.gpsimd.dma_start(out=pf16[:], in_=probs_d[i * P:(i + 1) * P, :])
        return pf16

    pf16s = [load_tile(0), load_tile(1)]

    # 2) targets: one [16, 128] contiguous load (HWDGE), PE-transpose to
    # [128, 16] so tile i's targets sit at column i as per-partition scalars.
    t_rows_i32 = consts.tile([NT, P], I32, tag="t_rows_i32")
    nc.sync.dma_start(out=t_rows_i32[:], in_=targ_d.rearrange("(i p) -> i p", p=P))

    # 3) constants
    ones = consts.tile([P, 1], F16, tag="ones")
    nc.vector.memset(ones[:], 1.0)
    iota_i16 = consts.tile([P, C], I16, tag="iota_i16")
    nc.gpsimd.iota(iota_i16[:], pattern=[[1, C]], base=0, channel_multiplier=0)
    iota_f16 = consts.tile([P, C], F16, tag="iota_f16")
    nc.vector.tensor_copy(iota_f16[:], iota_i16[:])
    identity = consts.tile([P, P], F32, tag="identity")
    make_identity(nc, identity[:])

    t_rows_f32 = consts.tile([NT, P], F32, tag="t_rows_f32")
    # gpsimd (not DVE) so the PE transpose below has single-engine producers
    nc.gpsimd.tensor_copy(t_rows_f32[:], t_rows_i32[:])
    t_ps = psum.tile([P, NT], F32, tag="t_ps")
    nc.tensor.transpose(t_ps[:], t_rows_f32[:], identity[:NT, :NT])
    t_cols = consts.tile([P, NT], F32, tag="t_cols")
    nc.vector.tensor_copy(t_cols[:], t_ps[:])

    # remaining probs loads
    pf16s += [load_tile(i) for i in range(2, NT)]

    # persistent accumulators
    cs_ps = [psum.tile([1, CH], F32, tag=f"cs_ps{h}", name=f"cs_ps{h}")
             for h in range(2)]
    hs_ps = [psum.tile([1, CH], F32, tag=f"hs_ps{h}", name=f"hs_ps{h}")
             for h in range(2)]
    pt_all = consts.tile([P, NT], F32, tag="pt_all")

    # 4a) all one-hot rows first — they depend only on iota/t_cols, so DVE
    # builds them while the probs DMAs stream in.
    eqs = []
    for i in range(NT):
        eq = eq_pool.tile([P, C], F16, tag="eq", name=f"eq_{i}")
        nc.vector.tensor_scalar(
            out=eq[:], in0=iota_f16[:], scalar1=t_cols[:, i:i + 1], scalar2=None,
            op0=Alu.is_equal,
        )
        eqs.append(eq)

    # 4b) all histogram matmuls as one dense DMA-independent block: ~13us of
    # back-to-back PE work early warms the HAM clock gate (2.4 GHz) before
    # the DMA-paced colsum matmuls arrive.
    for i in range(NT):
        first, last = (i == 0), (i == NT - 1)
        for h in range(2):
            sl = slice(h * CH, (h + 1) * CH)
            nc.tensor.matmul(hs_ps[h][:], ones[:], eqs[i][:, sl],
                             start=first, stop=last)

    # 4c) DMA-paced loop: colsum matmuls + pt extraction.
    for i in range(NT):
        pf16 = pf16s[i]
        first, last = (i == 0), (i == NT - 1)
        for h in range(2):
            sl = slice(h * CH, (h + 1) * CH)
            nc.tensor.matmul(cs_ps[h][:], ones[:], pf16[:, sl],
                             start=first, stop=last)

        # pt[p] = sum_c probs[p,c] * eq[p,c] — an exact one-element gather.
        # Split across engines to keep both under the DMA roofline: most
        # tiles go DVE-mult + ACT Copy-accumulate; the rest use the fused
        # DVE scalar_tensor_tensor reduce.
        if i % 4 < 3:
            msk = msk_pool.tile([P, C], F16, tag="msk", name=f"msk_{i}")
            nc.vector.tensor_tensor(out=msk[:], in0=eqs[i][:], in1=pf16[:],
                                    op=Alu.mult)
            ajunk = ajunk_pool.tile([P, C], F16, tag="ajunk")
            nc.scalar.activation(ajunk[:], msk[:],
                                 mybir.ActivationFunctionType.Copy,
                                 accum_out=pt_all[:, i:i + 1])
        else:
            msk = msk_pool.tile([P, C], F16, tag="msk", name=f"msk_{i}")
            nc.vector.scalar_tensor_tensor(
                out=msk[:], in0=iota_f16[:], scalar=t_cols[:, i:i + 1],
                in1=pf16[:], op0=Alu.is_equal, op1=Alu.mult,
                accum_out=pt_all[:, i:i + 1],
            )

    # 5) focal tail: focal[p] = sum_i (pt - 1) * ln(pt).
    # Stage [pt | ln(pt)] side by side via ACT so the DVE reduce depends on a
    # single engine.
    pl = consts.tile([P, 2 * NT], F32, tag="pl")
    nc.scalar.copy(pl[:, 0:NT], pt_all[:])
    nc.scalar.activation(pl[:, NT:2 * NT], pt_all[:],
                         mybir.ActivationFunctionType.Ln)
    junk = consts.tile([P, NT], F32, tag="junk")
    focal = consts.tile([P, 1], F32, tag="focal")
    nc.vector.scalar_tensor_tensor(
        out=junk[:], in0=pl[:, 0:NT], scalar=1.0, in1=pl[:, NT:2 * NT],
        op0=Alu.subtract, op1=Alu.mult, accum_out=focal[:],
    )
    # reduce focal over partitions: PE transpose to a row, ACT accumulates
    fc_t = psum.tile([1, P], F32, tag="fc_t")
    nc.tensor.transpose(fc_t[:], focal[:], identity[:])

    # 6) pack [colsum | hist | focal_sum] into one row, single output DMA.
    # colsum halves drain on DVE, hist halves on ACT — parallel tail.
    out_sb = consts.tile([1, OUT_W], F32, tag="out_sb")
    for h in range(2):
        nc.vector.tensor_copy(out_sb[:, h * CH:(h + 1) * CH], cs_ps[h][:])
        nc.scalar.copy(out_sb[:, C + h * CH:C + (h + 1) * CH], hs_ps[h][:])
    fc_row = consts.tile([1, P], F32, tag="fc_row")
    nc.scalar.activation(fc_row[:], fc_t[:],
                         mybir.ActivationFunctionType.Copy,
                         accum_out=out_sb[:, 2 * C:2 * C + 1])
    nc.sync.dma_start(out=out_d[:, :], in_=out_sb[:])


def _split_multi_waits(nc):
    """The walrus build in this env encodes at most ONE sync wait per
    instruction (newer Tile emits several, e.g. on its tail drain). Hoist
    extra waits onto EventSemaphore carrier instructions inserted just
    before, on the same engine — same-engine program order makes this
    semantically identical."""
    n = 0
    for f in nc.m.functions:
        for blk in f.blocks:
            il = blk.instructions
            i = 0
            while i < len(il):
                inst = il[i]
                si = inst.sync_info
                ws = list(si.on_wait) if si is not None else []
                if len(ws) > 1:
                    for w in ws[:-1]:
                        ev = mybir.InstEventSemaphore(
                            name=f"I-waitsplit-{n}", ins=[], outs=[])
                        n += 1
                        ev.engine = inst.engine
                        ev.sync_info = mybir.SyncInfo(on_wait=[w], on_update=[])
                        il.insert(i, ev)
                        i += 1
                    inst.sync_info = mybir.SyncInfo(
                        on_wait=[ws[-1]], on_update=list(si.on_update))
                i += 1


_cached_nc = {}


def build_nc(split_waits=True):
    global _cached_nc
    if split_waits in _cached_nc:
        return _cached_nc[split_waits]
    from contextlib import ExitStack

    nc = bass.Bass("TRN2")
    probs_d = nc.dram_tensor("probs", [BC, C], F32, kind="ExternalInput").ap()
    targ_d = nc.dram_tensor("targets", [BC], I32, kind="ExternalInput").ap()
    out_d = nc.dram_tensor("out_all", [1, OUT_W], F32, kind="ExternalOutput").ap()

    with tile.TileContext(nc) as tc:
        with ExitStack() as ctx:
            emit_kernel(ctx, tc, probs_d, targ_d, out_d)
    if split_waits:
        _split_multi_waits(nc)
    _cached_nc[split_waits] = nc
    return nc


def make_in_maps(probs, targets):
    probs = np.ascontiguousarray(np.asarray(probs), dtype=np.float32)
    targets = np.asarray(targets).astype(np.int32)
    assert probs.shape == (B, C) and targets.shape == (B,)
    return [
        {
            "probs": probs[k * BC:(k + 1) * BC],
            "targets": np.ascontiguousarray(targets[k * BC:(k + 1) * BC]),
        }
        for k in range(NCORES)
    ]


def combine(results):
    cs = np.zeros(C, np.float64)
    hs = np.zeros(C, np.float64)
    fc = 0.0
    for r in results:
        row = r["out_all"].reshape(OUT_W).astype(np.float64)
        cs += row[0:C]
        hs += row[C:2 * C]
        fc += row[2 * C]
    loss_cls = fc / B
    loss_cal = float(np.mean(np.abs(cs / B - hs / B)))
    return np.asarray(loss_cls + 1.0 * loss_cal, dtype=np.float32)


def run_spmd(probs, targets, **kwargs):
    nc = build_nc()
    in_maps = make_in_maps(probs, targets)
    return run_bass_kernel_spmd(nc, in_maps, list(range(NCORES)), **kwargs)


def kernel(probs, targets):
    res = run_spmd(probs, targets)
    return combine(res.results)


# revision 22
# speedup vs baseline: 1.2720x; 1.0028x over previous
"""FocalLoss + MDCA loss kernel for TRN2, 8-core data-parallel.

reference:
    loss_cls = mean_i[-(1-pt_i) * log(pt_i)],  pt_i = probs[i, targets[i]]
    loss_cal = mean_c |mean_i probs[i,c] - count_c/B|
    out = loss_cls + loss_cal        (GAMMA=1, BETA=1)

Strategy: shard batch (16384) across 8 cores (2048 rows each). Each core:
  - streams its probs shard HBM->SBUF with an inline fp32->fp16 cast (SWDGE)
    into dedicated per-tile buffers (no slot-reuse waits on the DMAs)
  - PE matmul ones[128,1]^T @ probs_fp16 accumulates column sums in PSUM (fp32)
  - DVE builds one-hot rows eq[p,c] = (c == target_p) from an iota constant,
    PE matmul ones^T @ eq accumulates the target histogram in PSUM (exact)
  - pt[p] = sum_c probs*eq (exact gather of the fp16-quantized prob), split
    across engines: most tiles DVE-mult + ACT Copy-accumulate, the rest a
    fused DVE scalar_tensor_tensor reduce
  - ACT stages [pt | ln(pt)], DVE fuses (pt-1)*ln(pt) with a row-sum, PE
    transposes the [128] focal partials and ACT reduces them to one scalar
  - everything lands in ONE [1, 2001] f32 output row -> a single contiguous
    DMA (per-partition 4B writes to DRAM pay a ~9us RMW receipt)
Host combines the 8 cores' colsum/hist/focal partials into the scalar loss
(the gather/unshard step).

The walrus build in this env encodes at most ONE sync wait per instruction;
_split_multi_waits post-processes the scheduled program to hoist extra waits
onto same-engine EventSemaphore carriers.
"""

import numpy as np

import concourse.bass as bass
import concourse.mybir as mybir
import concourse.tile as tile
from concourse.bass_utils import run_bass_kernel_spmd

B, C = 16384, 1000
NCORES = 8
BC = B // NCORES  # 2048 rows per core
P = 128
NT = BC // P      # 16 batch tiles per core
CH = 500          # matmul half free-dim (PSUM bank = 512 fp32)
OUT_W = 2001      # [colsum 0:1000 | hist 1000:2000 | focal_sum 2000]

F32 = mybir.dt.float32
F16 = mybir.dt.float16
I16 = mybir.dt.int16
I32 = mybir.dt.int32


def emit_kernel(ctx, tc, probs_d, targ_d, out_d):
    nc = tc.nc
    Alu = mybir.AluOpType
    from concourse.masks import make_identity

    consts = ctx.enter_context(tc.tile_pool(name="consts", bufs=1))
    probs_pool = ctx.enter_context(tc.tile_pool(name="probs_pool", bufs=NT))
    eq_pool = ctx.enter_context(tc.tile_pool(name="eq_pool", bufs=NT))
    psum = ctx.enter_context(tc.tile_pool(name="psum", bufs=1, space="PSUM"))

    # 1) first two probs tile loads start immediately (SDMA drains them while
    # POOL builds the constants); the rest are emitted right after.
    def load_tile(i):
        pf16 = probs_pool.tile([P, C], F16, tag="pf16", name=f"pf16_{i}")
        nc.gpsimd.dma_start(out=pf16[:], in_=probs_d[i * P:(i + 1) * P, :])
        return pf16

    pf16s = [load_tile(0), load_tile(1)]

    # 2) targets: one [16, 128] contiguous load (HWDGE), PE-transpose to
    # [128, 16] so tile i's targets sit at column i as per-partition scalars.
    t_rows_i32 = consts.tile([NT, P], I32, tag="t_rows_i32")
    nc.sync.dma_start(out=t_rows_i32[:], in_=targ_d.rearrange("(i p) -> i p", p=P))

    # 3) constants
    ones = consts.tile([P, 1], F16, tag="ones")
    nc.vector.memset(ones[:], 1.0)
    iota_i16 = consts.tile([P, C], I16, tag="iota_i16")
    nc.gpsimd.iota(iota_i16[:], pattern=[[1, C]], base=0, channel_multiplier=0)
    iota_f16 = consts.tile([P, C], F16, tag="iota_f16")
    nc.vector.tensor_copy(iota_f16[:], iota_i16[:])
    identity = consts.tile([P, P], F32, tag="identity")
    make_identity(nc, identity[:])

    t_rows_f32 = consts.tile([NT, P], F32, tag="t_rows_f32")
    # gpsimd (not DVE) so the PE transpose below has single-engine producers
    nc.gpsimd.tensor_copy(t_rows_f32[:], t_rows_i32[:])
    t_ps = psum.tile([P, NT], F32, tag="t_ps")
    nc.tensor.transpose(t_ps[:], t_rows_f32[:], identity[:NT, :NT])
    t_cols = consts.tile([P, NT], F32, tag="t_cols")
    nc.vector.tensor_copy(t_cols[:], t_ps[:])
    t_cols_i32 = consts.tile([P, NT], I32, tag="t_cols_i32")
    nc.vector.tensor_copy(t_cols_i32[:], t_ps[:])

    # pt[p, j] = probs[128j + p, t] in ONE indirect gather (exact fp32),
    # emitted before the bulk probs loads so its 2048 descriptors hit a
    # near-quiet ring (and the ring is 8x the default size).
    rows_i32 = consts.tile([P, NT], I32, tag="rows_i32")
    nc.gpsimd.iota(rows_i32[:], pattern=[[P, NT]], base=0, channel_multiplier=1)
    offs = consts.tile([P, NT], I32, tag="offs")
    nc.vector.tensor_scalar(out=offs[:], in0=rows_i32[:], scalar1=float(C),
                            scalar2=None, op0=Alu.mult)
    nc.vector.tensor_tensor(out=offs[:], in0=offs[:], in1=t_cols_i32[:],
                            op=Alu.add)
    pt_all = consts.tile([P, NT], F32, tag="pt_all")
    nc.gpsimd.indirect_dma_start(
        out=pt_all[:], out_offset=None,
        in_=probs_d.rearrange("a b -> (a b)")[:, None],
        in_offset=bass.IndirectOffsetOnAxis(ap=offs[:], axis=0),
    )

    # remaining probs loads
    pf16s += [load_tile(i) for i in range(2, NT)]

    # persistent accumulators
    cs_ps = [psum.tile([1, CH], F32, tag=f"cs_ps{h}", name=f"cs_ps{h}")
             for h in range(2)]
    hs_ps = [psum.tile([1, CH], F32, tag=f"hs_ps{h}", name=f"hs_ps{h}")
             for h in range(2)]

    # 4a) all one-hot rows first — they depend only on iota/t_cols, so DVE
    # builds them while the probs DMAs stream in.
    eqs = []
    for i in range(NT):
        eq = eq_pool.tile([P, C], F16, tag="eq", name=f"eq_{i}")
        nc.vector.tensor_scalar(
            out=eq[:], in0=iota_f16[:], scalar1=t_cols[:, i:i + 1], scalar2=None,
            op0=Alu.is_equal,
        )
        eqs.append(eq)

    # 4b) all histogram matmuls as one dense DMA-independent block: ~13us of
    # back-to-back PE work early warms the HAM clock gate (2.4 GHz) before
    # the DMA-paced colsum matmuls arrive.
    for i in range(NT):
        first, last = (i == 0), (i == NT - 1)
        for h in range(2):
            sl = slice(h * CH, (h + 1) * CH)
            nc.tensor.matmul(hs_ps[h][:], ones[:], eqs[i][:, sl],
                             start=first, stop=last)

    # 4c) DMA-paced loop: colsum matmuls + pt extraction.
    for i in range(NT):
        pf16 = pf16s[i]
        first, last = (i == 0), (i == NT - 1)
        for h in range(2):
            sl = slice(h * CH, (h + 1) * CH)
            nc.tensor.matmul(cs_ps[h][:], ones[:], pf16[:, sl],
                             start=first, stop=last)


    # 5) focal tail: focal[p] = sum_i (pt - 1) * ln(pt).
    # Stage [pt | ln(pt)] side by side via ACT so the DVE reduce depends on a
    # single engine.
    pl = consts.tile([P, 2 * NT], F32, tag="pl")
    nc.scalar.copy(pl[:, 0:NT], pt_all[:])
    nc.scalar.activation(pl[:, NT:2 * NT], pt_all[:],
                         mybir.ActivationFunctionType.Ln)
    junk = consts.tile([P, NT], F32, tag="junk")
    focal = consts.tile([P, 1], F32, tag="focal")
    nc.vector.scalar_tensor_tensor(
        out=junk[:], in0=pl[:, 0:NT], scalar=1.0, in1=pl[:, NT:2 * NT],
        op0=Alu.subtract, op1=Alu.mult, accum_out=focal[:],
    )
    # reduce focal over partitions: PE transpose to a row, ACT accumulates
    fc_t = psum.tile([1, P], F32, tag="fc_t")
    nc.tensor.transpose(fc_t[:], focal[:], identity[:])

    # 6) pack [colsum | hist | focal_sum] into one row, single output DMA.
    # colsum halves drain on DVE, hist halves on ACT — parallel tail.
    out_sb = consts.tile([1, OUT_W], F32, tag="out_sb")
    for h in range(2):
        nc.vector.tensor_copy(out_sb[:, h * CH:(h + 1) * CH], cs_ps[h][:])
        nc.scalar.copy(out_sb[:, C + h * CH:C + (h + 1) * CH], hs_ps[h][:])
    fc_row = consts.tile([1, P], F32, tag="fc_row")
    nc.scalar.activation(fc_row[:], fc_t[:],
                         mybir.ActivationFunctionType.Copy,
                         accum_out=out_sb[:, 2 * C:2 * C + 1])
    nc.sync.dma_start(out=out_d[:, :], in_=out_sb[:])


def _split_multi_waits(nc):
    """The walrus build in this env encodes at most ONE sync wait per
    instruction (newer Tile emits several, e.g. on its tail drain). Hoist
    extra waits onto EventSemaphore carrier instructions inserted just
    before, on the same engine — same-engine program order makes this
    semantically identical."""
    n = 0
    for f in nc.m.functions:
        for blk in f.blocks:
            il = blk.instructions
            i = 0
            while i < len(il):
                inst = il[i]
                si = inst.sync_info
                ws = list(si.on_wait) if si is not None else []
                if len(ws) > 1:
                    for w in ws[:-1]:
                        ev = mybir.InstEventSemaphore(
                            name=f"I-waitsplit-{n}", ins=[], outs=[])
                        n += 1
                        ev.engine = inst.engine
                        ev.sync_info = mybir.SyncInfo(on_wait=[w], on_update=[])
                        il.insert(i, ev)
                        i += 1
                    inst.sync_info = mybir.SyncInfo(
                        on_wait=[ws[-1]], on_update=list(si.on_update))
                i += 1


_cached_nc = {}


def build_nc(split_waits=True):
    global _cached_nc
    if split_waits in _cached_nc:
        return _cached_nc[split_waits]
    from contextlib import ExitStack

    nc = bass.Bass("TRN2", dynamic_dma_scratch_size=131072)
    probs_d = nc.dram_tensor("probs", [BC, C], F32, kind="ExternalInput").ap()
    targ_d = nc.dram_tensor("targets", [BC], I32, kind="ExternalInput").ap()
    out_d = nc.dram_tensor("out_all", [1, OUT_W], F32, kind="ExternalOutput").ap()

    with tile.TileContext(nc) as tc:
        with ExitStack() as ctx:
            emit_kernel(ctx, tc, probs_d, targ_d, out_d)
    if split_waits:
        _split_multi_waits(nc)
    _cached_nc[split_waits] = nc
    return nc


def make_in_maps(probs, targets):
    probs = np.ascontiguousarray(np.asarray(probs), dtype=np.float32)
    targets = np.asarray(targets).astype(np.int32)
    assert probs.shape == (B, C) and targets.shape == (B,)
    return [
        {
            "probs": probs[k * BC:(k + 1) * BC],
            "targets": np.ascontiguousarray(targets[k * BC:(k + 1) * BC]),
        }
        for k in range(NCORES)
    ]


def combine(results):
    cs = np.zeros(C, np.float64)
    hs = np.zeros(C, np.float64)
    fc = 0.0
    for r in results:
        row = r["out_all"].reshape(OUT_W).astype(np.float64)
        cs += row[0:C]
        hs += row[C:2 * C]
        fc += row[2 * C]
    loss_cls = fc / B
    loss_cal = float(np.mean(np.abs(cs / B - hs / B)))
    return np.asarray(loss_cls + 1.0 * loss_cal, dtype=np.float32)


def run_spmd(probs, targets, **kwargs):
    nc = build_nc()
    in_maps = make_in_maps(probs, targets)
    return run_bass_kernel_spmd(nc, in_maps, list(range(NCORES)), **kwargs)


def kernel(probs, targets):
    res = run_spmd(probs, targets)
    return combine(res.results)
